# revision 13
# baseline (speedup 1.0000x reference)
"""Bass/Trainium2 kernel for nn_BoxFilter: 9x9 circular box-mean over
(8, 3, 1024, 1024) f32, data-parallel across 8 NeuronCores (1 image/core).

All-bf16 I/O pipeline (rel err ~6e-3, gate is 2e-2), ~12.5 MB DMA/core:
  - host packs each image circularly padded: [3, 1036, 1032] bf16
    (rows/cols pre-wrapped, so no wraparound DMAs on device)
  - PE: v3 = vertical-9 x horizontal-3 sums via 3 column-shifted
    accumulating matmuls per PSUM chunk; one stationary band-weight
    matrix serves every block (k=128 overlap windows)
  - ACT: PSUM -> SBUF drain with x(1/81) scale, bf16 out
  - DVE: out[n] = u3[n] + u3[n+3] + u3[n+6] in two bf16 tensor_tensor
    passes (2x_1p mode), block-paired to halve instruction overhead
  - loads on the SP HWDGE ring (channel 0 split so compute starts
    early); stores issued per block-pair (480 KB), alternating between
    the ACT HWDGE ring and GPSIMD SWDGE so the tail drains fast
"""

import numpy as np
import ml_dtypes

import concourse.bacc as bacc
import concourse.mybir as mybir
import concourse.tile as tile
from concourse.ap import AP
from concourse.bass_utils import run_bass_kernel_spmd

B, C, H, W = 8, 3, 1024, 1024
R = 4             # filter radius
WIN = 2 * R + 1   # 9
AREA = WIN * WIN  # 81
M = 120           # output rows per full block (input window = 128 rows)
NB = 8            # full blocks per channel
MT = H - NB * M   # 64: tail block output rows
KT = MT + 2 * R   # 72: tail block input rows
RPAD = H + R + 8  # 1036 padded rows: padded row i == real row (i-4) mod H
CP = W + 2 * R    # 1032 padded cols: padded col j == real col (j-4) mod W
U3 = CP - 2       # 1030 u3 columns per block
# PSUM chunking: each matmul output must stay within one 512-f32 PSUM bank
CHUNKS = ((0, 512), (512, 512), (1024, U3 - 1024))

_CACHE: dict = {}


def _band_weights() -> np.ndarray:
    w = np.zeros((128, M), dtype=ml_dtypes.bfloat16)
    for m in range(M):
        w[m : m + WIN, m] = 1.0
    return w


def _pack_image(x: np.ndarray) -> np.ndarray:
    """[C,H,W] f32 -> [C, 1036, 1032] bf16, circularly padded by R=4
    (rows: 4 top / 8 bottom, cols: 4 each side)."""
    rows = (np.arange(RPAD) - R) % H
    cols = (np.arange(CP) - R) % W
    xp = x[:, rows][:, :, cols]
    return np.ascontiguousarray(xp.astype(ml_dtypes.bfloat16))


def _build():
    f32 = mybir.dt.float32
    bf16 = mybir.dt.bfloat16
    add = mybir.AluOpType.add
    nc = bacc.Bacc("TRN2", target_bir_lowering=False, debug=False, num_devices=B)
    x_d = nc.dram_tensor("x", [C, RPAD, CP], bf16, kind="ExternalInput")
    w_d = nc.dram_tensor("w", [128, M], bf16, kind="ExternalInput")
    o_d = nc.dram_tensor("o", [C, H, W], bf16, kind="ExternalOutput")
    XCH = RPAD * CP  # elements per packed channel
    OCH = H * W      # elements per output channel

    with tile.TileContext(nc) as tc:
        with (
            tc.tile_pool(name="wpool", bufs=1) as wpool,
            tc.tile_pool(name="xpool", bufs=3) as xpool,
            tc.tile_pool(name="upool", bufs=3) as upool,
            tc.tile_pool(name="tpool", bufs=2) as tpool,
            tc.tile_pool(name="opool", bufs=2) as opool,
            tc.tile_pool(name="psum", bufs=2, space="PSUM") as psum,
        ):
            QB = 4  # blocks per u3/combine group
            w_t = wpool.tile([128, M], bf16)
            nc.sync.dma_start(w_t[:], w_d.ap())

            def load_channel(c, split_first):
                x_t = xpool.tile([128, NB + 1, CP], bf16, tag="x")
                # windows 0..7: partition p, window w <- padded row 120w + p
                if split_first:
                    # first window on the ACT ring: overlaps the w load on sync
                    nc.scalar.dma_start(
                        x_t[0:128, 0:1, :], AP(x_d, c * XCH, [[CP, 128], [1, CP]])
                    )
                    nc.sync.dma_start(
                        x_t[0:128, 1:NB, :],
                        AP(x_d, c * XCH + M * CP, [[CP, 128], [M * CP, NB - 1], [1, CP]]),
                    )
                else:
                    nc.sync.dma_start(
                        x_t[0:128, 0:NB, :],
                        AP(x_d, c * XCH, [[CP, 128], [M * CP, NB], [1, CP]]),
                    )
                # tail window: padded rows 960..1031 (72 rows)
                nc.sync.dma_start(
                    x_t[0:KT, NB, :],
                    AP(x_d, c * XCH + NB * M * CP, [[CP, KT], [1, CP]]),
                )
                return x_t

            def vertical3(x_t, j, m, k):
                """9 matmuls: v3[mm, i] = sum_{d=0..2} sum_kk band(kk,mm) x[kk, j, i+d]."""
                v3_t = psum.tile([128, U3], f32, tag="v3")
                for c0, cn in CHUNKS:
                    for d in range(3):
                        nc.tensor.matmul(
                            v3_t[0:m, c0 : c0 + cn],
                            w_t[0:k, 0:m],
                            x_t[0:k, j, c0 + d : c0 + d + cn],
                            start=(d == 0),
                            stop=(d == 2),
                        )
                return v3_t

            def drain(v3_t, u3_t, q, m):
                nc.scalar.mul(
                    out=u3_t[0:m, q, 0:U3], in_=v3_t[0:m, 0:U3], mul=1.0 / AREA
                )

            def combine(u3_t, o_t, j0, nq, m, q0=0, gp_half=False):
                """out[n] = u3[n] + u3[n+3] + u3[n+6] over nq stacked blocks.
                gp_half: second half of pass-2 runs on GPSIMD."""
                t_t = tpool.tile([128, QB, CP], bf16, tag="t")
                nc.vector.tensor_tensor(
                    out=t_t[0:m, 0:nq, 0:W],
                    in0=u3_t[0:m, q0 : q0 + nq, 0:W],
                    in1=u3_t[0:m, q0 : q0 + nq, 3 : W + 3],
                    op=add,
                )
                h = nq // 2 if (gp_half and nq > 1) else nq
                nc.vector.tensor_tensor(
                    out=o_t[0:m, j0 : j0 + h, :],
                    in0=t_t[0:m, 0:h, 0:W],
                    in1=u3_t[0:m, q0 : q0 + h, 6:U3],
                    op=add,
                )
                if h < nq:
                    nc.gpsimd.tensor_tensor(
                        out=o_t[0:m, j0 + h : j0 + nq, :],
                        in0=t_t[0:m, h:nq, 0:W],
                        in1=u3_t[0:m, q0 + h : q0 + nq, 6:U3],
                        op=add,
                    )

            def store_rows(c, o_t, j0, nj, eng):
                eng.dma_start(
                    AP(o_d, c * OCH + j0 * M * W, [[W, M], [M * W, nj], [1, W]]),
                    o_t[0:M, j0 : j0 + nj, :],
                )

            def do_tail(c, x_t, o_t):
                u3_t = upool.tile([128, QB, CP], bf16, tag="u3")
                v3_t = vertical3(x_t, NB, MT, KT)
                drain(v3_t, u3_t, 0, MT)
                combine(u3_t, o_t, NB, 1, MT)
                nc.gpsimd.dma_start(
                    AP(o_d, c * OCH + NB * M * W, [[W, MT], [1, W]]),
                    o_t[0:MT, NB, :],
                )

            def do_quad(c, x_t, o_t, g, gp_half):
                u3_t = upool.tile([128, QB, CP], bf16, tag="u3")
                for q in range(QB):
                    v3_t = vertical3(x_t, QB * g + q, M, 128)
                    drain(v3_t, u3_t, q, M)
                combine(u3_t, o_t, QB * g, QB, M, gp_half=gp_half)
                store_rows(c, o_t, QB * g, QB, nc.sync if g == 0 else nc.gpsimd)

            x_tiles = [load_channel(c, split_first=(c == 0)) for c in range(C)]
            for c in range(C - 1):
                x_t = x_tiles[c]
                o_t = opool.tile([128, NB + 1, W], bf16, tag="o")
                do_quad(c, x_t, o_t, 0, gp_half=False)
                do_quad(c, x_t, o_t, 1, gp_half=True)
                do_tail(c, x_t, o_t)
            # last channel: tail first, then shrinking groups -> short pipe tail
            c = C - 1
            x_t = x_tiles[c]
            o_t = opool.tile([128, NB + 1, W], bf16, tag="o")
            do_tail(c, x_t, o_t)
            do_quad(c, x_t, o_t, 0, gp_half=True)
            u3_t = upool.tile([128, QB, CP], bf16, tag="u3")
            for q in range(2):
                v3_t = vertical3(x_t, 4 + q, M, 128)
                drain(v3_t, u3_t, q, M)
            combine(u3_t, o_t, 4, 2, M)
            store_rows(c, o_t, 4, 2, nc.gpsimd)
            u3_t = upool.tile([128, QB, CP], bf16, tag="u3")
            for q in range(2):
                v3_t = vertical3(x_t, 6 + q, M, 128)
                drain(v3_t, u3_t, q, M)
                combine(u3_t, o_t, 6 + q, 1, M, q0=q)
            store_rows(c, o_t, 6, 2, nc.sync)
    nc.compile()
    return nc


def _get_nc():
    if "nc" not in _CACHE:
        _CACHE["nc"] = _build()
    return _CACHE["nc"]


def _prepare_in_maps(tensor: np.ndarray) -> list:
    x = np.asarray(tensor, dtype=np.float32)
    assert x.shape == (B, C, H, W), x.shape
    wmat = _band_weights()
    return [{"x": _pack_image(x[i]), "w": wmat} for i in range(B)]


def kernel(tensor: np.ndarray) -> np.ndarray:
    nc = _get_nc()
    in_maps = _prepare_in_maps(tensor)
    res = run_bass_kernel_spmd(nc, in_maps, core_ids=list(range(B)))
    return np.stack(
        [res.results[i]["o"].astype(np.float32) for i in range(B)], axis=0
    )
